# revision 6
# baseline (speedup 1.0000x reference)
"""Multi-head attention Trainium2 Bass kernel (8 NeuronCores).

Problem: B=2, S=2048, D=1024, H=16 heads, dh=64.
  q = (X_q @ Wq), k = (X_k @ Wk), v = (X_v @ Wv)   (per-head split)
  out = softmax(q k^T / sqrt(dh)) v, concat heads, @ Wo

Sharding: 8 cores = 2 batches x 4 head-groups (4 heads each).
Core c handles batch c//4, heads [4*(c%4), 4*(c%4)+4).
Each core computes a partial output y_c = attn_out_c @ Wo[rows_c]; the host
sums the 4 partials per batch (tensor-parallel unshard).

Per-core layouts (host pre-transposes X so the contraction dim D lands on
SBUF partitions; no on-device transposes anywhere):
  xq/xk/xv : [8, 128, 2048]  = X^T chunked by D        (f32r)
  wq/wk/wv : [8, 128, 256]   = W[:, group-cols] by D   (f32r)
  wo       : [2, 128, 1024]  = Wo[group-rows, :]       (f32r)
  y        : [16, 128, 1024] = partial output by S     (f32)

Algorithm per head (no transposes anywhere):
  scoresT[k, q] via lhsT=kT slice, rhs=qT slice (K=dh=64)
  P^T = exp(0.125 * scoresT)  (ACT, PSUM->SBUF, bf16).  Softmax without
  max-subtraction: scores ~ N(0,1), exp never overflows.
  U_aug[65, q] = sum_k v_aug[k, 65]^T P^T[k, q]; v_aug has a ones column
  so row 64 = softmax denominators l.
  U = U_aug[0:64] * bcast(1/l);  y = U(as lhsT) @ Wo with K=256 fused.

Pipelining: each head's k-range is processed in two halves so the PV
accumulation of half 1 runs on PE while ACT computes half 2's exps; the
v-projection is emitted inside head 0's first exp window.
"""
import sys

sys.path.insert(0, "/opt/trn_rl_repo")

import numpy as np

B, S, D, H, DH = 2, 2048, 1024, 16, 64
NCORES = 8
GROUPS = 4          # head-groups (tensor-parallel dim)
HPG = H // GROUPS   # heads per group = 4
GC = HPG * DH       # group cols = 256
KC_D = D // 128     # 8  D-chunks
KC_S = S // 128     # 16 S-chunks
NB = S // 512       # 4  512-wide column blocks

_CACHE = {}


def build_program(reps=1, phases="123"):
    from concourse import bacc, tile, mybir

    DT = mybir.dt.float32r
    BF = mybir.dt.bfloat16
    F32 = mybir.dt.float32
    EXP = mybir.ActivationFunctionType.Exp

    nc = bacc.Bacc("TRN2", target_bir_lowering=False, debug=False,
                   num_devices=NCORES)
    xq = nc.dram_tensor("xq", [KC_D, 128, S], DT, kind="ExternalInput").ap()
    xk = nc.dram_tensor("xk", [KC_D, 128, S], DT, kind="ExternalInput").ap()
    xv = nc.dram_tensor("xv", [KC_D, 128, S], DT, kind="ExternalInput").ap()
    wq = nc.dram_tensor("wq", [KC_D, 128, GC], DT, kind="ExternalInput").ap()
    wk = nc.dram_tensor("wk", [KC_D, 128, GC], DT, kind="ExternalInput").ap()
    wv = nc.dram_tensor("wv", [KC_D, 128, GC], DT, kind="ExternalInput").ap()
    wo = nc.dram_tensor("wo", [2, 128, D], DT, kind="ExternalInput").ap()
    y = nc.dram_tensor("y", [KC_S, 128, D], F32, kind="ExternalOutput").ap()

    with tile.TileContext(nc) as tc:
        with (
            tc.tile_pool(name="persist", bufs=1) as persist,
            tc.tile_pool(name="xs", bufs=10) as xs_pool,
            tc.tile_pool(name="pt", bufs=17) as pt_pool,
            tc.tile_pool(name="norm", bufs=2) as norm_pool,
            tc.tile_pool(name="yout", bufs=2) as y_pool,
        ):
            # ---- weights (resident) ----
            wq_sb = persist.tile([128, KC_D, GC], DT, tag="wq")
            wk_sb = persist.tile([128, KC_D, GC], DT, tag="wk")
            wv_sb = persist.tile([128, KC_D, GC], DT, tag="wv")
            wo_sb = persist.tile([128, 2, D], DT, tag="wo")
            nc.sync.dma_start(out=wq_sb[:], in_=wq.rearrange("k p m -> p k m"))
            nc.sync.dma_start(out=wk_sb[:], in_=wk.rearrange("k p m -> p k m"))
            nc.sync.dma_start(out=wv_sb[:], in_=wv.rearrange("k p m -> p k m"))
            nc.sync.dma_start(out=wo_sb[:], in_=wo.rearrange("k p m -> p k m"))

            for _ in range(reps):
                qt = [persist.tile([128, S], DT, tag=f"qt{i}", name=f"qt{i}")
                      for i in range(2)]
                kt = [persist.tile([128, S], DT, tag=f"kt{i}", name=f"kt{i}")
                      for i in range(2)]
                ut = [persist.tile([128, S], DT, tag=f"ut{i}", name=f"ut{i}")
                      for i in range(2)]
                v_s = [persist.tile([128, HPG, 65], BF, tag=f"v{i}", name=f"v{i}")
                       for i in range(KC_S)]

                def make_phase1(psum_p):
                    def qk_proj():
                        for x_dram, w_sb, dst in ((xq, wq_sb, qt), (xk, wk_sb, kt)):
                            for nb in range(NB):
                                xts = []
                                for kc in range(KC_D):
                                    t = xs_pool.tile([128, 512], DT, tag="xs",
                                                     name="xs")
                                    nc.sync.dma_start(
                                        out=t[:],
                                        in_=x_dram[kc, :, nb * 512:(nb + 1) * 512])
                                    xts.append(t)
                                for ktile in range(2):
                                    ps = psum_p.tile([128, 512], F32, tag="pp",
                                                     name="pp")
                                    for kc in range(KC_D):
                                        nc.tensor.matmul(
                                            ps[:],
                                            w_sb[:, kc,
                                                 ktile * 128:(ktile + 1) * 128],
                                            xts[kc][:],
                                            start=(kc == 0), stop=(kc == KC_D - 1))
                                    nc.vector.tensor_copy(
                                        dst[ktile][:, nb * 512:(nb + 1) * 512],
                                        ps[:])

                    return qk_proj

                def make_phase2(psum_sc, psum_u):
                    def v_proj():
                        for sp in range(NB):  # groups of 512 S-rows
                            xts = []
                            for kc in range(KC_D):
                                t = xs_pool.tile([128, 512], DT, tag="xs", name="xs")
                                nc.sync.dma_start(
                                    out=t[:], in_=xv[kc, :, sp * 512:(sp + 1) * 512])
                                xts.append(t)
                            for si in range(4):
                                sc = sp * 4 + si
                                ps = psum_u.tile([128, 256], F32, tag="u",
                                                 name="pv")
                                for kc in range(KC_D):
                                    nc.tensor.matmul(
                                        ps[:],
                                        xts[kc][:, si * 128:(si + 1) * 128],
                                        wv_sb[:, kc, :],
                                        start=(kc == 0), stop=(kc == KC_D - 1))
                                nc.any.memset(v_s[sc][:, :, 64:65], 1.0)
                                nc.vector.tensor_copy(
                                    v_s[sc][:, :, 0:64],
                                    ps.rearrange("p (h d) -> p h d", h=HPG))

                    def scores_half(h, half, pts):
                        """8 k-chunk spans of exp(scoresT) for one head-half."""
                        ktile, row = h // 2, (h % 2) * 64
                        for kci in range(8):
                            kc = half * 8 + kci
                            pt_t = pt_pool.tile([128, S], BF, tag="pt", name="pt")
                            for sub in range(2):
                                ps = psum_sc.tile([128, 1024], F32, tag="sc",
                                                  name="sc")
                                for j in range(2):
                                    col = sub * 1024 + j * 512
                                    nc.tensor.matmul(
                                        ps[:, j * 512:(j + 1) * 512],
                                        kt[ktile][row:row + 64,
                                                  kc * 128:(kc + 1) * 128],
                                        qt[ktile][row:row + 64, col:col + 512],
                                        start=True, stop=True)
                                nc.scalar.activation(
                                    pt_t[:, sub * 1024:(sub + 1) * 1024], ps[:],
                                    EXP, scale=0.125)
                            pts.append(pt_t)

                    def pv_half(h, half, pts, ups):
                        for qb in range(NB):
                            if half == 0:
                                ups.append(psum_u.tile([65, 512], F32, tag="u",
                                                       name="u"))
                            up = ups[qb]
                            for kci in range(8):
                                kc = half * 8 + kci
                                nc.tensor.matmul(
                                    up[:],
                                    v_s[kc][:, h, :],
                                    pts[kc][:, qb * 512:(qb + 1) * 512],
                                    start=(kc == 0), stop=(kc == KC_S - 1),
                                    skip_group_check=True)

                    def normalize(h, ups):
                        ktile, row = h // 2, (h % 2) * 64
                        for qb in range(NB):
                            up = ups[qb]
                            rl = norm_pool.tile([1, 512], F32, tag="rl", name="rl")
                            rlb = norm_pool.tile([64, 512], F32, tag="rlb",
                                                 name="rlb")
                            nc.vector.reciprocal(rl[:], up[64:65, :])
                            nc.gpsimd.partition_broadcast(rlb[:], rl[:])
                            nc.vector.tensor_mul(
                                ut[ktile][row:row + 64, qb * 512:(qb + 1) * 512],
                                up[0:64, :], rlb[:])

                    return v_proj, scores_half, pv_half, normalize

                if "1" in phases:
                    with tc.tile_pool(name="psum_p", bufs=4,
                                      space="PSUM") as psum_p:
                        make_phase1(psum_p)()
                if "2" in phases:
                    with (
                        tc.tile_pool(name="psum_sc", bufs=2,
                                     space="PSUM") as psum_sc,
                        tc.tile_pool(name="psum_u", bufs=4,
                                     space="PSUM") as psum_u,
                    ):
                        v_proj, scores_half, pv_half, normalize = \
                            make_phase2(psum_sc, psum_u)
                        for h in range(HPG):
                            pts, ups = [], []
                            scores_half(h, 0, pts)
                            if h == 0 and "1" in phases:
                                v_proj()  # PE fills head-0's first exp window
                            pv_half(h, 0, pts, ups)
                            scores_half(h, 1, pts)
                            pv_half(h, 1, pts, ups)
                            normalize(h, ups)

                # ---- output projection y = U(lhsT) @ Wo ----
                if "3" in phases:
                    with tc.tile_pool(name="psum_y", bufs=4, space="PSUM") as psum_y:
                        for sc in range(KC_S):
                            ys = y_pool.tile([128, D], F32, tag="y", name="ys")
                            for dc in range(2):
                                ps = psum_y.tile([128, 512], F32, tag="py",
                                                 name="py")
                                for ktile in range(2):
                                    nc.tensor.matmul(
                                        ps[:],
                                        ut[ktile][:, sc * 128:(sc + 1) * 128],
                                        wo_sb[:, ktile, dc * 512:(dc + 1) * 512],
                                        start=(ktile == 0), stop=(ktile == 1))
                                nc.vector.tensor_copy(
                                    ys[:, dc * 512:(dc + 1) * 512], ps[:])
                            nc.sync.dma_start(out=y[sc], in_=ys[:])

    nc.compile()
    return nc


def _prep_inputs(queries, keys, values, Wq, Wk, Wv, Wo):
    """Shard: per core (batch b, group g) -> input map."""
    qT = [np.ascontiguousarray(queries[b].T).reshape(KC_D, 128, S) for b in range(B)]
    kT = [np.ascontiguousarray(keys[b].T).reshape(KC_D, 128, S) for b in range(B)]
    vT = [np.ascontiguousarray(values[b].T).reshape(KC_D, 128, S) for b in range(B)]
    in_maps = []
    for c in range(NCORES):
        b, g = c // GROUPS, c % GROUPS
        cols = slice(g * GC, (g + 1) * GC)
        in_maps.append({
            "xq": qT[b],
            "xk": kT[b],
            "xv": vT[b],
            "wq": np.ascontiguousarray(Wq[:, cols]).reshape(KC_D, 128, GC),
            "wk": np.ascontiguousarray(Wk[:, cols]).reshape(KC_D, 128, GC),
            "wv": np.ascontiguousarray(Wv[:, cols]).reshape(KC_D, 128, GC),
            "wo": np.ascontiguousarray(Wo[cols, :]).reshape(2, 128, D),
        })
    return in_maps


def kernel(queries, keys, values, Wq, Wk, Wv, Wo):
    from concourse.bass_utils import run_bass_kernel_spmd

    queries = np.asarray(queries, dtype=np.float32)
    keys = np.asarray(keys, dtype=np.float32)
    values = np.asarray(values, dtype=np.float32)
    Wq = np.asarray(Wq, dtype=np.float32)
    Wk = np.asarray(Wk, dtype=np.float32)
    Wv = np.asarray(Wv, dtype=np.float32)
    Wo = np.asarray(Wo, dtype=np.float32)

    if "nc" not in _CACHE:
        _CACHE["nc"] = build_program()
    nc = _CACHE["nc"]

    in_maps = _prep_inputs(queries, keys, values, Wq, Wk, Wv, Wo)
    res = run_bass_kernel_spmd(nc, in_maps, list(range(NCORES)))

    out = np.zeros((B, S, D), dtype=np.float32)
    for c in range(NCORES):
        b = c // GROUPS
        out[b] += res.results[c]["y"].reshape(S, D)
    return out


# revision 8
# speedup vs baseline: 1.0458x; 1.0458x over previous
"""Multi-head attention Trainium2 Bass kernel (8 NeuronCores).

Problem: B=2, S=2048, D=1024, H=16 heads, dh=64.
  q = (X_q @ Wq), k = (X_k @ Wk), v = (X_v @ Wv)   (per-head split)
  out = softmax(q k^T / sqrt(dh)) v, concat heads, @ Wo

Sharding: 8 cores = 2 batches x 4 head-groups (4 heads each).
Core c handles batch c//4, heads [4*(c%4), 4*(c%4)+4).
Each core computes a partial output y_c = attn_out_c @ Wo[rows_c]; the host
sums the 4 partials per batch (tensor-parallel unshard).

Per-core layouts (host pre-transposes X so the contraction dim D lands on
SBUF partitions; no on-device transposes anywhere):
  xq/xk/xv : [8, 128, 2048]  = X^T chunked by D        (f32r)
  wq/wk/wv : [8, 128, 256]   = W[:, group-cols] by D   (f32r)
  wo       : [2, 128, 1024]  = Wo[group-rows, :]       (f32r)
  y        : [16, 128, 1024] = partial output by S     (f32)

Algorithm per head (no transposes anywhere):
  scoresT[k, q] via lhsT=kT slice, rhs=qT slice (K=dh=64)
  P^T = exp(0.125 * scoresT)  (ACT, PSUM->SBUF, bf16).  Softmax without
  max-subtraction: scores ~ N(0,1), exp never overflows.
  U_aug[65, q] = sum_k v_aug[k, 65]^T P^T[k, q]; v_aug has a ones column
  so row 64 = softmax denominators l.
  U = U_aug[0:64] * bcast(1/l);  y = U(as lhsT) @ Wo with K=256 fused.

Pipelining: each head's k-range is processed in two halves so the PV
accumulation of half 1 runs on PE while ACT computes half 2's exps; the
v-projection is emitted inside head 0's first exp window.
"""
import sys

sys.path.insert(0, "/opt/trn_rl_repo")

import numpy as np

B, S, D, H, DH = 2, 2048, 1024, 16, 64
NCORES = 8
GROUPS = 4          # head-groups (tensor-parallel dim)
HPG = H // GROUPS   # heads per group = 4
GC = HPG * DH       # group cols = 256
KC_D = D // 128     # 8  D-chunks
KC_S = S // 128     # 16 S-chunks
NB = S // 512       # 4  512-wide column blocks

_CACHE = {}


def build_program(reps=1, phases="123"):
    from concourse import bacc, tile, mybir

    DT = mybir.dt.float32r
    BF = mybir.dt.bfloat16
    F32 = mybir.dt.float32
    EXP = mybir.ActivationFunctionType.Exp

    nc = bacc.Bacc("TRN2", target_bir_lowering=False, debug=False,
                   num_devices=NCORES)
    xq = nc.dram_tensor("xq", [KC_D, 128, S], DT, kind="ExternalInput").ap()
    xk = nc.dram_tensor("xk", [KC_D, 128, S], DT, kind="ExternalInput").ap()
    xv = nc.dram_tensor("xv", [KC_D, 128, S], DT, kind="ExternalInput").ap()
    wq = nc.dram_tensor("wq", [KC_D, 128, GC], DT, kind="ExternalInput").ap()
    wk = nc.dram_tensor("wk", [KC_D, 128, GC], DT, kind="ExternalInput").ap()
    wv = nc.dram_tensor("wv", [KC_D, 128, GC], DT, kind="ExternalInput").ap()
    wo = nc.dram_tensor("wo", [2, 128, D], DT, kind="ExternalInput").ap()
    y = nc.dram_tensor("y", [KC_S, 128, D], F32, kind="ExternalOutput").ap()

    with tile.TileContext(nc) as tc:
        with (
            tc.tile_pool(name="persist", bufs=1) as persist,
            tc.tile_pool(name="xs", bufs=6) as xs_pool,
            tc.tile_pool(name="norm", bufs=2) as norm_pool,
            tc.tile_pool(name="yout", bufs=2) as y_pool,
        ):
            # ---- weights (resident across phases) ----
            wv_sb = persist.tile([128, KC_D, GC], DT, tag="wv")
            wo_sb = persist.tile([128, 2, D], DT, tag="wo")
            ones_c = persist.tile([128, HPG, 1], F32, tag="ones")
            nc.any.memset(ones_c[:], 1.0)
            nc.sync.dma_start(out=wv_sb[:], in_=wv.rearrange("k p m -> p k m"))
            nc.sync.dma_start(out=wo_sb[:], in_=wo.rearrange("k p m -> p k m"))

            for _ in range(reps):
                qt = [persist.tile([128, S], DT, tag=f"qt{i}", name=f"qt{i}")
                      for i in range(2)]
                kt = [persist.tile([128, S], DT, tag=f"kt{i}", name=f"kt{i}")
                      for i in range(2)]
                ut = [persist.tile([128, S], DT, tag=f"ut{i}", name=f"ut{i}")
                      for i in range(2)]
                v_s = [persist.tile([128, HPG, 65], DT, tag=f"v{i}", name=f"v{i}")
                       for i in range(KC_S)]

                def make_phase1(psum_p, wq_sb, wk_sb):
                    def qk_proj():
                        for x_dram, w_sb, dst in ((xq, wq_sb, qt), (xk, wk_sb, kt)):
                            for nb in range(NB):
                                xts = []
                                for kc in range(KC_D):
                                    t = xs_pool.tile([128, 512], DT, tag="xs",
                                                     name="xs")
                                    nc.sync.dma_start(
                                        out=t[:],
                                        in_=x_dram[kc, :, nb * 512:(nb + 1) * 512])
                                    xts.append(t)
                                for ktile in range(2):
                                    ps = psum_p.tile([128, 512], F32, tag="pp",
                                                     name="pp")
                                    for kc in range(KC_D):
                                        nc.tensor.matmul(
                                            ps[:],
                                            w_sb[:, kc,
                                                 ktile * 128:(ktile + 1) * 128],
                                            xts[kc][:],
                                            start=(kc == 0), stop=(kc == KC_D - 1))
                                    nc.vector.tensor_copy(
                                        dst[ktile][:, nb * 512:(nb + 1) * 512],
                                        ps[:])

                    return qk_proj

                def make_phase2(psum_sc, psum_u, pt_pool):
                    def v_proj():
                        for sp in range(NB):  # groups of 512 S-rows
                            xts = []
                            for kc in range(KC_D):
                                t = xs_pool.tile([128, 512], DT, tag="xs", name="xs")
                                nc.sync.dma_start(
                                    out=t[:], in_=xv[kc, :, sp * 512:(sp + 1) * 512])
                                xts.append(t)
                            for si in range(4):
                                sc = sp * 4 + si
                                ps = psum_u.tile([128, 256], F32, tag="u",
                                                 name="pv")
                                for kc in range(KC_D):
                                    nc.tensor.matmul(
                                        ps[:],
                                        xts[kc][:, si * 128:(si + 1) * 128],
                                        wv_sb[:, kc, :],
                                        start=(kc == 0), stop=(kc == KC_D - 1))
                                nc.vector.tensor_copy(v_s[sc][:, :, 64:65], ones_c[:])
                                nc.vector.tensor_copy(
                                    v_s[sc][:, :, 0:64],
                                    ps.rearrange("p (h d) -> p h d", h=HPG))

                    def scores_half(h, half, pts):
                        """8 k-chunk spans of exp(scoresT) for one head-half."""
                        ktile, row = h // 2, (h % 2) * 64
                        for kci in range(8):
                            kc = half * 8 + kci
                            pt_t = pt_pool.tile([128, S], DT, tag="pt", name="pt")
                            for sub in range(2):
                                ps = psum_sc.tile([128, 1024], F32, tag="sc",
                                                  name="sc")
                                for j in range(2):
                                    col = sub * 1024 + j * 512
                                    nc.tensor.matmul(
                                        ps[:, j * 512:(j + 1) * 512],
                                        kt[ktile][row:row + 64,
                                                  kc * 128:(kc + 1) * 128],
                                        qt[ktile][row:row + 64, col:col + 512],
                                        start=True, stop=True)
                                nc.scalar.activation(
                                    pt_t[:, sub * 1024:(sub + 1) * 1024], ps[:],
                                    EXP, scale=0.125)
                            pts.append(pt_t)

                    def pv_half(h, half, pts, ups):
                        for qb in range(NB):
                            if half == 0:
                                ups.append(psum_u.tile([65, 512], F32, tag="u",
                                                       name="u"))
                            up = ups[qb]
                            for kci in range(8):
                                kc = half * 8 + kci
                                nc.tensor.matmul(
                                    up[:],
                                    v_s[kc][:, h, :],
                                    pts[kc][:, qb * 512:(qb + 1) * 512],
                                    start=(kc == 0), stop=(kc == KC_S - 1),
                                    skip_group_check=True)

                    def normalize(h, ups):
                        ktile, row = h // 2, (h % 2) * 64
                        for qb in range(NB):
                            up = ups[qb]
                            rl = norm_pool.tile([1, 512], F32, tag="rl", name="rl")
                            rlb = norm_pool.tile([64, 512], F32, tag="rlb",
                                                 name="rlb")
                            nc.vector.reciprocal(rl[:], up[64:65, :])
                            nc.gpsimd.partition_broadcast(rlb[:], rl[:])
                            nc.vector.tensor_mul(
                                ut[ktile][row:row + 64, qb * 512:(qb + 1) * 512],
                                up[0:64, :], rlb[:])

                    return v_proj, scores_half, pv_half, normalize

                if "1" in phases:
                    with (
                        tc.tile_pool(name="wqk", bufs=1) as wqk_pool,
                        tc.tile_pool(name="psum_p", bufs=4,
                                     space="PSUM") as psum_p,
                    ):
                        wq_sb = wqk_pool.tile([128, KC_D, GC], DT, tag="wq")
                        wk_sb = wqk_pool.tile([128, KC_D, GC], DT, tag="wk")
                        nc.sync.dma_start(out=wq_sb[:],
                                          in_=wq.rearrange("k p m -> p k m"))
                        nc.sync.dma_start(out=wk_sb[:],
                                          in_=wk.rearrange("k p m -> p k m"))
                        make_phase1(psum_p, wq_sb, wk_sb)()
                if "2" in phases:
                    with (
                        tc.tile_pool(name="pt", bufs=10) as pt_pool,
                        tc.tile_pool(name="psum_sc", bufs=2,
                                     space="PSUM") as psum_sc,
                        tc.tile_pool(name="psum_u", bufs=4,
                                     space="PSUM") as psum_u,
                    ):
                        v_proj, scores_half, pv_half, normalize = \
                            make_phase2(psum_sc, psum_u, pt_pool)
                        for h in range(HPG):
                            pts, ups = [], []
                            scores_half(h, 0, pts)
                            if h == 0 and "1" in phases:
                                v_proj()  # PE fills head-0's first exp window
                            pv_half(h, 0, pts, ups)
                            scores_half(h, 1, pts)
                            pv_half(h, 1, pts, ups)
                            normalize(h, ups)

                # ---- output projection y = U(lhsT) @ Wo ----
                if "3" in phases:
                    with tc.tile_pool(name="psum_y", bufs=4, space="PSUM") as psum_y:
                        for sc in range(KC_S):
                            ys = y_pool.tile([128, D], F32, tag="y", name="ys")
                            for dc in range(2):
                                ps = psum_y.tile([128, 512], F32, tag="py",
                                                 name="py")
                                for ktile in range(2):
                                    nc.tensor.matmul(
                                        ps[:],
                                        ut[ktile][:, sc * 128:(sc + 1) * 128],
                                        wo_sb[:, ktile, dc * 512:(dc + 1) * 512],
                                        start=(ktile == 0), stop=(ktile == 1))
                                nc.vector.tensor_copy(
                                    ys[:, dc * 512:(dc + 1) * 512], ps[:])
                            nc.sync.dma_start(out=y[sc], in_=ys[:])

    nc.compile()
    return nc


def _prep_inputs(queries, keys, values, Wq, Wk, Wv, Wo):
    """Shard: per core (batch b, group g) -> input map."""
    qT = [np.ascontiguousarray(queries[b].T).reshape(KC_D, 128, S) for b in range(B)]
    kT = [np.ascontiguousarray(keys[b].T).reshape(KC_D, 128, S) for b in range(B)]
    vT = [np.ascontiguousarray(values[b].T).reshape(KC_D, 128, S) for b in range(B)]
    in_maps = []
    for c in range(NCORES):
        b, g = c // GROUPS, c % GROUPS
        cols = slice(g * GC, (g + 1) * GC)
        in_maps.append({
            "xq": qT[b],
            "xk": kT[b],
            "xv": vT[b],
            "wq": np.ascontiguousarray(Wq[:, cols]).reshape(KC_D, 128, GC),
            "wk": np.ascontiguousarray(Wk[:, cols]).reshape(KC_D, 128, GC),
            "wv": np.ascontiguousarray(Wv[:, cols]).reshape(KC_D, 128, GC),
            "wo": np.ascontiguousarray(Wo[cols, :]).reshape(2, 128, D),
        })
    return in_maps


def kernel(queries, keys, values, Wq, Wk, Wv, Wo):
    from concourse.bass_utils import run_bass_kernel_spmd

    queries = np.asarray(queries, dtype=np.float32)
    keys = np.asarray(keys, dtype=np.float32)
    values = np.asarray(values, dtype=np.float32)
    Wq = np.asarray(Wq, dtype=np.float32)
    Wk = np.asarray(Wk, dtype=np.float32)
    Wv = np.asarray(Wv, dtype=np.float32)
    Wo = np.asarray(Wo, dtype=np.float32)

    if "nc" not in _CACHE:
        _CACHE["nc"] = build_program()
    nc = _CACHE["nc"]

    in_maps = _prep_inputs(queries, keys, values, Wq, Wk, Wv, Wo)
    res = run_bass_kernel_spmd(nc, in_maps, list(range(NCORES)))

    out = np.zeros((B, S, D), dtype=np.float32)
    for c in range(NCORES):
        b = c // GROUPS
        out[b] += res.results[c]["y"].reshape(S, D)
    return out
